# revision 29
# baseline (speedup 1.0000x reference)
"""Trainium2 Bass kernel for block-diagonal (per-frame) multi-head attention.

Reference semantics (fp32):
    q = x@Wq + bq ; k = x@Wk + bk ; v = relu(x@Wv + bv)   (per head, d_head=32)
    scores = (q k^T) / sqrt(32) within each 24-token frame, -inf across frames
    attn = softmax(scores) with +1e-6 in the denominator
    out = attn @ v

Mapping: 16 batches data-parallel over 8 cores (2 batches/core).  Within a
core: x is transposed via PE (bf16), projections produce qT/kT in
[d_model, token] layout and V in [token, d_model] layout (96-token partition
tiles).  Attention runs per (batch, head, group-of-4-frames): one K=32 matmul
produces the dense 96x96 score block (transposed orientation [k, q]), exp is
applied on ScalarE (no max subtraction: scores are bounded, and skipping the
max only perturbs the +eps term by <1e-6 relative), the block-diagonal mask
is applied as a bf16 multiply on VectorE, and the AV matmul uses a
ones-augmented V so the softmax denominator comes out as column 32 of the
same matmul.  Normalization then happens on the [96, 12, 33] PSUM output
with per-partition scalars.
"""

import math
import os
from contextlib import ExitStack

import numpy as np

import concourse.bass as bass
from concourse import bacc
import concourse.mybir as mybir
import concourse.tile as tile
from concourse.bass_utils import run_bass_kernel_spmd
from concourse.masks import make_block_diagonal, make_identity

F32 = mybir.dt.float32
BF16 = mybir.dt.bfloat16
AF = mybir.ActivationFunctionType
ALU = mybir.AluOpType

BS = 16
SEQ = 48
J = 24           # joints (tokens per frame)
N_TOK = SEQ * J  # 1152 tokens per batch
D_IN = 256
H = 8
DH = 32
DM = 256
N_CORES = 8
B2 = BS // N_CORES          # batches per core
TOK = B2 * N_TOK            # 2304 tokens per core
G = 96                      # tokens per attention group (4 frames)
FPG = G // J                # 4 frames per group
NG = TOK // G               # 24 groups per core
NGB = N_TOK // G            # 12 groups per batch
WAVE = 6                    # groups per exp wave; [96, 6, 128] f32 = 2 PSUM banks
SCALE = 1.0 / math.sqrt(DH)
EPS = 1e-6

_CACHE = {}


def _build(skip=(), trace_sim=False):
    nc = bacc.Bacc(trn_type="TRN2")

    x_d = nc.dram_tensor("x", [TOK, D_IN], F32, kind="ExternalInput")
    wq_d = nc.dram_tensor("Wq", [D_IN, DM], F32, kind="ExternalInput")
    wk_d = nc.dram_tensor("Wk", [D_IN, DM], F32, kind="ExternalInput")
    wv_d = nc.dram_tensor("Wv", [D_IN, DM], F32, kind="ExternalInput")
    bq_d = nc.dram_tensor("bq", [DM], F32, kind="ExternalInput")
    bk_d = nc.dram_tensor("bk", [DM], F32, kind="ExternalInput")
    bv_d = nc.dram_tensor("bv", [DM], F32, kind="ExternalInput")
    out_d = nc.dram_tensor("out", [TOK, DM], F32, kind="ExternalOutput")

    with tile.TileContext(nc, trace_sim=trace_sim) as tc, ExitStack() as ctx:
        singles = ctx.enter_context(tc.tile_pool(name="singles", bufs=1))
        tpps = ctx.enter_context(tc.tile_pool(name="tpps", bufs=2, space="PSUM"))
        mmps = ctx.enter_context(tc.tile_pool(name="mmps", bufs=2, space="PSUM"))
        scps = ctx.enter_context(tc.tile_pool(name="scps", bufs=2, space="PSUM"))
        avps = tpps  # av tiles share the transpose pool's two banks
        epool = ctx.enter_context(tc.tile_pool(name="epool", bufs=4))
        rpool = ctx.enter_context(tc.tile_pool(name="rpool", bufs=4))

        # ---- constants ----
        ident = singles.tile([128, 128], BF16)
        make_identity(nc, ident)

        # weights: [128 (d_in within half), 2 (d_in half), 256] bf16.
        # Cast on ScalarE (idle until the first projection): keeping GpSimd
        # free lets the first x-tile cast start as soon as its DMA lands.
        w_bf = []
        for wd in (wq_d, wk_d, wv_d):
            wf = singles.tile([128, 2, DM], F32, tag=f"wf_{wd.name}")
            nc.sync.dma_start(wf, wd[:].rearrange("(a p) m -> p a m", p=128))
            wb = singles.tile([128, 2, DM], BF16, tag=f"w_{wd.name}")
            nc.scalar.copy(wb, wf)
            w_bf.append(wb)
        wq_bf, wk_bf, wv_bf = w_bf

        # q/k biases as per-partition scalars: [128, 2 halves] fp32
        bq_sb = singles.tile([128, 2], F32, tag="bq")
        nc.sync.dma_start(bq_sb, bq_d[:].rearrange("(a p) -> p a", p=128))
        bk_sb = singles.tile([128, 2], F32, tag="bk")
        nc.sync.dma_start(bk_sb, bk_d[:].rearrange("(a p) -> p a", p=128))
        # v bias rides into the PSUM via a K=1 ones matmul (bias along the
        # free axis can't use the per-partition ScalarE bias path)
        bv_f = singles.tile([1, DM], F32, tag="bv_f")
        nc.sync.dma_start(bv_f, bv_d[None, :])
        bv_bf = singles.tile([1, DM], BF16, tag="bv_bf")
        nc.scalar.copy(bv_bf, bv_f)
        ones_col = singles.tile([1, G], BF16, tag="ones_col")
        nc.vector.memset(ones_col, 1.0)

        # block-diagonal 0/1 mask for one 4-frame group, bf16 [96, 96]
        mask = singles.tile([G, G], BF16, tag="mask")
        make_block_diagonal(nc, mask, J)

        # ---- persistent activations ----
        xT = singles.tile([128, 2, TOK], BF16, tag="xT")
        qT = singles.tile([128, 2, TOK], BF16, tag="qT")
        kT = singles.tile([128, 2, TOK], BF16, tag="kT")
        # V augmented with a ones column: [96, group, head, 33]
        v_aug = singles.tile([G, NG, H, DH + 1], BF16, tag="vaug")
        nc.vector.memset(v_aug[:, :, :, DH:DH + 1], 1.0)
        out_sb = singles.tile([G, NG, DM], F32, tag="out")

        # ---- attention ----
        out_view = out_d[:].rearrange("(g p) c -> p g c", p=G)

        # sc PSUM layout [96, WAVE, 128]: the 128-slot stride keeps every
        # matmul output (96 cols used) inside one 2 KB PSUM bank.
        # Emitted as a function so batch 0 can be issued mid-projection (its
        # inputs are complete after chunk 2), keeping the PE queue dense
        # across the projection -> attention boundary.
        def emit_attention(b):
            for h in range(H):
                half = h // 4
                hr = (h % 4) * 32
                av = avps.tile([G, NGB, DH + 1], F32, tag="tp")
                for w in range(NGB // WAVE):
                    sc = scps.tile([G, WAVE, 128], F32, tag="sc")
                    for gl in range(WAVE):
                        g = w * WAVE + gl
                        c0 = b * N_TOK + g * G
                        nc.tensor.matmul(
                            sc[:, gl, :G],
                            lhsT=kT[hr:hr + 32, half, c0:c0 + G],
                            rhs=qT[hr:hr + 32, half, c0:c0 + G],
                            start=True,
                            stop=True,
                            tile_position=(hr, 0),
                        )
                    eT = epool.tile([G, WAVE, G], BF16, tag="eT")
                    nc.scalar.activation(
                        out=eT, in_=sc[:, :, :G], func=AF.Exp, scale=SCALE
                    )
                    eTm = epool.tile([G, WAVE, G], BF16, tag="eTm")
                    # Alternate the mask multiply between VectorE and GpSimd
                    # to balance the two engines' load.
                    mask_eng = nc.vector if w % 2 == 0 else nc.gpsimd
                    mask_eng.tensor_tensor(
                        eTm,
                        eT,
                        mask[:, None, :].to_broadcast((G, WAVE, G)),
                        ALU.mult,
                    )
                    for gl in range(WAVE):
                        g = w * WAVE + gl
                        nc.tensor.matmul(
                            av[:, g, :],
                            lhsT=eTm[:, gl, :],
                            rhs=v_aug[:, b * NGB + g, h, :],
                            start=True,
                            stop=True,
                        )
                # normalize: out = av[:, :, :32] / (av[:, :, 32] + eps)
                rt = rpool.tile([G, NGB], F32, tag="rt")
                nc.vector.tensor_scalar_add(rt, av[:, :, DH], EPS)
                nc.vector.reciprocal(rt, rt)
                nc.vector.tensor_tensor(
                    out_sb[:, b * NGB:(b + 1) * NGB, h * DH:(h + 1) * DH],
                    av[:, :, 0:DH],
                    rt[:, :, None].to_broadcast((G, NGB, DH)),
                    ALU.mult,
                )
                # store this (batch, head) slice as soon as it is normalized
                nc.sync.dma_start(
                    out_view[:, b * NGB:(b + 1) * NGB, h * DH:(h + 1) * DH],
                    out_sb[:, b * NGB:(b + 1) * NGB, h * DH:(h + 1) * DH],
                )


        # ---- load + cast + transpose x, interleaved with projections ----
        # x lives whole in SBUF (18.4 KB/partition fp32); chunked DMAs into
        # disjoint slices of one tensor avoid DMA slot-reuse waits (the
        # DIRECT2D DMA lowering only supports 2 sync waits).
        NT = TOK // 128  # 18 token tiles
        x_f32 = singles.tile([128, NT, D_IN], F32, tag="x_f32")
        x_bf = singles.tile([128, NT, D_IN], BF16, tag="x_bf")
        x_view = x_d[:].rearrange("(t p) d -> p t d", p=128)  # [128, 18, 256]
        # First two tiles land as single-tile DMAs so the cast -> transpose
        # pipeline starts as early as possible; the rest come in pairs.
        x_dma_chunks = [(0, 1), (1, 1)] + [(t, 2) for t in range(2, NT, 2)]
        for t0, tn in x_dma_chunks:
            nc.sync.dma_start(
                x_f32[:, t0:t0 + tn, :], x_view[:, t0:t0 + tn, :]
            )

        # Emission is interleaved per 512-token chunk (4 x-tiles): casts ->
        # transposes -> q/k projection matmuls -> v projection for the groups
        # the chunk completes.  This keeps the PE queue dense from ~1 us on,
        # so the HAM clock gate warms once (~3.4 us) and stays warm, instead
        # of the PE idling 18 us behind a serial gpsimd cast chain and
        # running throttled at 1.2 GHz until 35 us.
        # Casts rotate gpsimd/vector/gpsimd/scalar (gpsimd is ~2x slower but
        # has spare capacity here); relus alternate vector/scalar (gpsimd has
        # no PSUM port).
        cast_ops = [
            lambda o, i: nc.gpsimd.tensor_copy(o, i),
            lambda o, i: nc.vector.tensor_copy(o, i),
            lambda o, i: nc.gpsimd.tensor_copy(o, i),
            lambda o, i: nc.scalar.copy(o, i),
        ]
        relu_ops = [
            lambda o, i: nc.vector.tensor_scalar_max(o, i, 0.0),
            lambda o, i: nc.scalar.activation(out=o, in_=i, func=AF.Relu),
        ]
        chunk_tiles = [(0, 4), (4, 4), (8, 4), (12, 4), (16, 2)]
        chunks = [(c, min(512, TOK - c)) for c in range(0, TOK, 512)]
        gdone = 0  # v-proj groups emitted so far
        for ci, (tile0, ntile) in enumerate(
            chunk_tiles if "proj" not in skip else []
        ):
            # casts + transposes for this chunk's tiles; two transposes share
            # one [128, 256] PSUM tile and one DVE copy.
            for tp0 in range(tile0, tile0 + ntile, 2):
                for t in (tp0, tp0 + 1):
                    cast_ops[t % 4](x_bf[:, t, :], x_f32[:, t, :])
                for a in range(2):
                    tp = tpps.tile([128, 256], BF16, tag="tp")
                    for dt in range(2):
                        nc.tensor.transpose(
                            tp[:, 128 * dt:128 * (dt + 1)],
                            x_bf[:, tp0 + dt, a * 128:(a + 1) * 128],
                            ident,
                        )
                    nc.vector.tensor_copy(
                        xT[:, a, tp0 * 128:(tp0 + 2) * 128], tp
                    )
            # q/k projection for this chunk
            c0, cn = chunks[ci]
            for half in range(2):
                for dst, wb, b_sb in ((qT, wq_bf, bq_sb), (kT, wk_bf, bk_sb)):
                    ps = mmps.tile([128, 512], F32, tag="proj")
                    for kk in range(2):
                        nc.tensor.matmul(
                            ps[:, :cn],
                            lhsT=wb[:, kk, half * 128:(half + 1) * 128],
                            rhs=xT[:, kk, c0:c0 + cn],
                            start=(kk == 0),
                            stop=(kk == 1),
                        )
                    nc.scalar.activation(
                        out=dst[:, half, c0:c0 + cn],
                        in_=ps[:, :cn],
                        func=AF.Identity,
                        bias=b_sb[:, half:half + 1],
                        scale=1.0,
                    )
            # v projection for every group fully covered by tokens so far
            gready = min(NG, (c0 + cn) // G)
            for g in range(gdone, gready):
                ps_full = mmps.tile([128, 512], F32, tag="proj", name="vproj")
                ps = ps_full[:G, :DM]
                for kk in range(2):
                    nc.tensor.matmul(
                        ps,
                        lhsT=xT[:, kk, g * G:(g + 1) * G],
                        rhs=wv_bf[:, kk, :],
                        start=(kk == 0),
                        stop=False,
                    )
                nc.tensor.matmul(
                    ps, lhsT=ones_col, rhs=bv_bf, start=False, stop=True
                )
                relu_ops[g % 2](
                    v_aug[:, g, :, 0:DH],
                    ps.rearrange("p (h d) -> p h d", h=H),
                )
            gdone = gready
            if ci == 2 and "attn" not in skip:
                # batch 0 inputs (tokens < 1152, v groups < 12) are
                # complete: issue its attention now so the PE queue
                # stays dense while batch 1 projections finish.
                emit_attention(0)

        if "attn" not in skip:
            emit_attention(1)

    nc.compile()
    return nc


def _get_nc():
    if "nc" not in _CACHE:
        _CACHE["nc"] = _build()
    return _CACHE["nc"]


def _run(inputs, **kw):
    nc = _get_nc()
    x = np.ascontiguousarray(inputs["x"], dtype=np.float32)
    shared = {
        k: np.ascontiguousarray(inputs[k], dtype=np.float32)
        for k in ("Wq", "Wk", "Wv", "bq", "bk", "bv")
    }
    in_maps = []
    for c in range(N_CORES):
        m = dict(shared)
        m["x"] = np.ascontiguousarray(
            x[c * B2:(c + 1) * B2].reshape(TOK, D_IN)
        )
        in_maps.append(m)
    res = run_bass_kernel_spmd(nc, in_maps, core_ids=list(range(N_CORES)), **kw)
    out = np.concatenate(
        [r["out"].reshape(B2, N_TOK, DM) for r in res.results], axis=0
    )
    return out, res


def kernel(**inputs) -> np.ndarray:
    out, _ = _run(inputs)
    return out



# revision 35
# speedup vs baseline: 1.0730x; 1.0730x over previous
"""Trainium2 Bass kernel for block-diagonal (per-frame) multi-head attention.

Reference semantics (fp32):
    q = x@Wq + bq ; k = x@Wk + bk ; v = relu(x@Wv + bv)   (per head, d_head=32)
    scores = (q k^T) / sqrt(32) within each 24-token frame, -inf across frames
    attn = softmax(scores) with +1e-6 in the denominator
    out = attn @ v

Mapping: 16 batches data-parallel over 8 cores (2 batches/core).  Within a
core: x is transposed via PE (bf16), projections produce qT/kT in
[d_model, token] layout and V in [token, d_model] layout (96-token partition
tiles).  Attention runs per (batch, head, group-of-4-frames): one K=32 matmul
produces the dense 96x96 score block (transposed orientation [k, q]), exp is
applied on ScalarE (no max subtraction: scores are bounded, and skipping the
max only perturbs the +eps term by <1e-6 relative), the block-diagonal mask
is applied as a bf16 multiply on VectorE, and the AV matmul uses a
ones-augmented V so the softmax denominator comes out as column 32 of the
same matmul.  Normalization then happens on the [96, 12, 33] PSUM output
with per-partition scalars.
"""

import math
import os
from contextlib import ExitStack

import numpy as np

import concourse.bass as bass
from concourse import bacc
import concourse.mybir as mybir
import concourse.tile as tile
from concourse.bass_utils import run_bass_kernel_spmd
from concourse.masks import make_block_diagonal, make_identity

F32 = mybir.dt.float32
BF16 = mybir.dt.bfloat16
AF = mybir.ActivationFunctionType
ALU = mybir.AluOpType

BS = 16
SEQ = 48
J = 24           # joints (tokens per frame)
N_TOK = SEQ * J  # 1152 tokens per batch
D_IN = 256
H = 8
DH = 32
DM = 256
N_CORES = 8
B2 = BS // N_CORES          # batches per core
TOK = B2 * N_TOK            # 2304 tokens per core
G = 96                      # tokens per attention group (4 frames)
FPG = G // J                # 4 frames per group
NG = TOK // G               # 24 groups per core
NGB = N_TOK // G            # 12 groups per batch
WAVE = 6                    # groups per exp wave; [96, 6, 128] f32 = 2 PSUM banks
SCALE = 1.0 / math.sqrt(DH)
EPS = 1e-6

_CACHE = {}


def _build(skip=(), trace_sim=False):
    nc = bacc.Bacc(trn_type="TRN2")

    x_d = nc.dram_tensor("x", [TOK, D_IN], F32, kind="ExternalInput")
    wq_d = nc.dram_tensor("Wq", [D_IN, DM], F32, kind="ExternalInput")
    wk_d = nc.dram_tensor("Wk", [D_IN, DM], F32, kind="ExternalInput")
    wv_d = nc.dram_tensor("Wv", [D_IN, DM], F32, kind="ExternalInput")
    bq_d = nc.dram_tensor("bq", [DM], F32, kind="ExternalInput")
    bk_d = nc.dram_tensor("bk", [DM], F32, kind="ExternalInput")
    bv_d = nc.dram_tensor("bv", [DM], F32, kind="ExternalInput")
    out_d = nc.dram_tensor("out", [TOK, DM], F32, kind="ExternalOutput")

    with tile.TileContext(nc, trace_sim=trace_sim) as tc, ExitStack() as ctx:
        singles = ctx.enter_context(tc.tile_pool(name="singles", bufs=1))
        tpps = ctx.enter_context(tc.tile_pool(name="tpps", bufs=2, space="PSUM"))
        mmps = ctx.enter_context(tc.tile_pool(name="mmps", bufs=2, space="PSUM"))
        scps = ctx.enter_context(tc.tile_pool(name="scps", bufs=2, space="PSUM"))
        avps = tpps  # av tiles share the transpose pool's two banks
        epool = ctx.enter_context(tc.tile_pool(name="epool", bufs=4))
        rpool = ctx.enter_context(tc.tile_pool(name="rpool", bufs=4))

        # ---- constants ----
        ident = singles.tile([128, 128], BF16)
        make_identity(nc, ident)

        # weights: [128 (d_in within half), 2 (d_in half), 256] bf16.
        # Cast on ScalarE (idle until the first projection): keeping GpSimd
        # free lets the first x-tile cast start as soon as its DMA lands.
        w_bf = []
        for wd in (wq_d, wk_d, wv_d):
            wf = singles.tile([128, 2, DM], F32, tag=f"wf_{wd.name}")
            nc.sync.dma_start(wf, wd[:].rearrange("(a p) m -> p a m", p=128))
            wb = singles.tile([128, 2, DM], BF16, tag=f"w_{wd.name}")
            nc.scalar.copy(wb, wf)
            w_bf.append(wb)
        wq_bf, wk_bf, wv_bf = w_bf

        # q/k biases as per-partition scalars: [128, 2 halves] fp32
        bq_sb = singles.tile([128, 2], F32, tag="bq")
        nc.sync.dma_start(bq_sb, bq_d[:].rearrange("(a p) -> p a", p=128))
        bk_sb = singles.tile([128, 2], F32, tag="bk")
        nc.sync.dma_start(bk_sb, bk_d[:].rearrange("(a p) -> p a", p=128))
        # v bias rides into the PSUM via a K=1 ones matmul (bias along the
        # free axis can't use the per-partition ScalarE bias path)
        bv_f = singles.tile([1, DM], F32, tag="bv_f")
        nc.sync.dma_start(bv_f, bv_d[None, :])
        bv_bf = singles.tile([1, DM], BF16, tag="bv_bf")
        nc.scalar.copy(bv_bf, bv_f)
        ones_col = singles.tile([1, G], BF16, tag="ones_col")
        nc.vector.memset(ones_col, 1.0)

        # block-diagonal 0/1 mask for one 4-frame group, bf16 [96, 96]
        mask = singles.tile([G, G], BF16, tag="mask")
        make_block_diagonal(nc, mask, J)

        # ---- persistent activations ----
        xT = singles.tile([128, 2, TOK], BF16, tag="xT")
        qT = singles.tile([128, 2, TOK], BF16, tag="qT")
        kT = singles.tile([128, 2, TOK], BF16, tag="kT")
        # V augmented with a ones column: [96, group, head, 33]
        v_aug = singles.tile([G, NG, H, DH + 1], BF16, tag="vaug")
        nc.vector.memset(v_aug[:, :, :, DH:DH + 1], 1.0)
        out_sb = singles.tile([G, NG, DM], F32, tag="out")

        # ---- load + cast + transpose x, interleaved with projections ----
        # x lives whole in SBUF (18.4 KB/partition fp32); chunked DMAs into
        # disjoint slices of one tensor avoid DMA slot-reuse waits (the
        # DIRECT2D DMA lowering only supports 2 sync waits).
        NT = TOK // 128  # 18 token tiles
        x_f32 = singles.tile([128, NT, D_IN], F32, tag="x_f32")
        x_bf = singles.tile([128, NT, D_IN], BF16, tag="x_bf")
        x_view = x_d[:].rearrange("(t p) d -> p t d", p=128)  # [128, 18, 256]
        # First two tiles land as single-tile DMAs so the cast -> transpose
        # pipeline starts as early as possible; the rest come in pairs.
        x_dma_chunks = [(0, 1), (1, 1)] + [(t, 2) for t in range(2, NT, 2)]
        for t0, tn in x_dma_chunks:
            nc.sync.dma_start(
                x_f32[:, t0:t0 + tn, :], x_view[:, t0:t0 + tn, :]
            )

        # Emission is interleaved per 512-token chunk (4 x-tiles): casts ->
        # transposes -> q/k projection matmuls -> v projection for the groups
        # the chunk completes.  This keeps the PE queue dense from ~1 us on,
        # so the HAM clock gate warms once (~3.4 us) and stays warm, instead
        # of the PE idling 18 us behind a serial gpsimd cast chain and
        # running throttled at 1.2 GHz until 35 us.
        # Casts rotate gpsimd/vector/gpsimd/scalar (gpsimd is ~2x slower but
        # has spare capacity here); relus alternate vector/scalar (gpsimd has
        # no PSUM port).
        cast_ops = [
            lambda o, i: nc.gpsimd.tensor_copy(o, i),
            lambda o, i: nc.vector.tensor_copy(o, i),
            lambda o, i: nc.gpsimd.tensor_copy(o, i),
            lambda o, i: nc.scalar.copy(o, i),
        ]
        relu_ops = [
            lambda o, i: nc.vector.tensor_scalar_max(o, i, 0.0),
            lambda o, i: nc.scalar.activation(out=o, in_=i, func=AF.Relu),
        ]
        chunk_tiles = [(0, 4), (4, 4), (8, 4), (12, 4), (16, 2)]
        chunks = [(c, min(512, TOK - c)) for c in range(0, TOK, 512)]
        gdone = 0  # v-proj groups emitted so far
        for ci, (tile0, ntile) in enumerate(
            chunk_tiles if "proj" not in skip else []
        ):
            # casts + transposes for this chunk's tiles; two transposes share
            # one [128, 256] PSUM tile and one DVE copy.
            for tp0 in range(tile0, tile0 + ntile, 2):
                for t in (tp0, tp0 + 1):
                    cast_ops[t % 4](x_bf[:, t, :], x_f32[:, t, :])
                for a in range(2):
                    tp = tpps.tile([128, 256], BF16, tag="tp")
                    for dt in range(2):
                        nc.tensor.transpose(
                            tp[:, 128 * dt:128 * (dt + 1)],
                            x_bf[:, tp0 + dt, a * 128:(a + 1) * 128],
                            ident,
                        )
                    nc.vector.tensor_copy(
                        xT[:, a, tp0 * 128:(tp0 + 2) * 128], tp
                    )
            # q/k projection for this chunk
            c0, cn = chunks[ci]
            for half in range(2):
                for dst, wb, b_sb in ((qT, wq_bf, bq_sb), (kT, wk_bf, bk_sb)):
                    ps = mmps.tile([128, 512], F32, tag="proj")
                    for kk in range(2):
                        nc.tensor.matmul(
                            ps[:, :cn],
                            lhsT=wb[:, kk, half * 128:(half + 1) * 128],
                            rhs=xT[:, kk, c0:c0 + cn],
                            start=(kk == 0),
                            stop=(kk == 1),
                        )
                    nc.scalar.activation(
                        out=dst[:, half, c0:c0 + cn],
                        in_=ps[:, :cn],
                        func=AF.Identity,
                        bias=b_sb[:, half:half + 1],
                        scale=1.0,
                    )
            # v projection for every group fully covered by tokens so far
            gready = min(NG, (c0 + cn) // G)
            for g in range(gdone, gready):
                ps_full = mmps.tile([128, 512], F32, tag="proj", name="vproj")
                ps = ps_full[:G, :DM]
                for kk in range(2):
                    nc.tensor.matmul(
                        ps,
                        lhsT=xT[:, kk, g * G:(g + 1) * G],
                        rhs=wv_bf[:, kk, :],
                        start=(kk == 0),
                        stop=False,
                    )
                nc.tensor.matmul(
                    ps, lhsT=ones_col, rhs=bv_bf, start=False, stop=True
                )
                relu_ops[g % 2](
                    v_aug[:, g, :, 0:DH],
                    ps.rearrange("p (h d) -> p h d", h=H),
                )
            gdone = gready

        # ---- attention ----
        out_view = out_d[:].rearrange("(g p) c -> p g c", p=G)
        # sc PSUM layout [96, WAVE, 128]: the 128-slot stride keeps every
        # matmul output (96 cols used) inside one 2 KB PSUM bank.
        for b in range(B2 if "attn" not in skip else 0):
            for h in range(H):
                half = h // 4
                hr = (h % 4) * 32
                av = avps.tile([G, NGB, DH + 1], F32, tag="tp")
                for w in range(NGB // WAVE):
                    sc = scps.tile([G, WAVE, 128], F32, tag="sc")
                    for gl in range(WAVE):
                        g = w * WAVE + gl
                        c0 = b * N_TOK + g * G
                        nc.tensor.matmul(
                            sc[:, gl, :G],
                            lhsT=kT[hr:hr + 32, half, c0:c0 + G],
                            rhs=qT[hr:hr + 32, half, c0:c0 + G],
                            start=True,
                            stop=True,
                            tile_position=(hr, 0),
                        )
                    eT = epool.tile([G, WAVE, G], BF16, tag="eT")
                    nc.scalar.activation(
                        out=eT, in_=sc[:, :, :G], func=AF.Exp, scale=SCALE
                    )
                    eTm = epool.tile([G, WAVE, G], BF16, tag="eTm")
                    # Alternate the mask multiply between VectorE and GpSimd
                    # to balance the two engines' load.
                    mask_eng = nc.vector if w % 2 == 0 else nc.gpsimd
                    mask_eng.tensor_tensor(
                        eTm,
                        eT,
                        mask[:, None, :].to_broadcast((G, WAVE, G)),
                        ALU.mult,
                    )
                    for gl in range(WAVE):
                        g = w * WAVE + gl
                        nc.tensor.matmul(
                            av[:, g, :],
                            lhsT=eTm[:, gl, :],
                            rhs=v_aug[:, b * NGB + g, h, :],
                            start=True,
                            stop=True,
                        )
                # normalize: out = av[:, :, :32] / (av[:, :, 32] + eps)
                rt = rpool.tile([G, NGB], F32, tag="rt")
                nc.vector.tensor_scalar_add(rt, av[:, :, DH], EPS)
                nc.vector.reciprocal(rt, rt)
                nc.vector.tensor_tensor(
                    out_sb[:, b * NGB:(b + 1) * NGB, h * DH:(h + 1) * DH],
                    av[:, :, 0:DH],
                    rt[:, :, None].to_broadcast((G, NGB, DH)),
                    ALU.mult,
                )
                # store this (batch, head) slice as soon as it is normalized
                nc.sync.dma_start(
                    out_view[:, b * NGB:(b + 1) * NGB, h * DH:(h + 1) * DH],
                    out_sb[:, b * NGB:(b + 1) * NGB, h * DH:(h + 1) * DH],
                )

    nc.compile()
    return nc


def _get_nc():
    if "nc" not in _CACHE:
        _CACHE["nc"] = _build()
    return _CACHE["nc"]


def _run(inputs, **kw):
    nc = _get_nc()
    x = np.ascontiguousarray(inputs["x"], dtype=np.float32)
    shared = {
        k: np.ascontiguousarray(inputs[k], dtype=np.float32)
        for k in ("Wq", "Wk", "Wv", "bq", "bk", "bv")
    }
    in_maps = []
    for c in range(N_CORES):
        m = dict(shared)
        m["x"] = np.ascontiguousarray(
            x[c * B2:(c + 1) * B2].reshape(TOK, D_IN)
        )
        in_maps.append(m)
    res = run_bass_kernel_spmd(nc, in_maps, core_ids=list(range(N_CORES)), **kw)
    out = np.concatenate(
        [r["out"].reshape(B2, N_TOK, DM) for r in res.results], axis=0
    )
    return out, res


def kernel(**inputs) -> np.ndarray:
    out, _ = _run(inputs)
    return out

